# revision 42
# baseline (speedup 1.0000x reference)
"""Trainium2 Bass kernel for nn_LinearRecurrenceLayer.

Reference computation (per batch row, L=4096, D=1024):
    norm = ||x_l|| / sqrt(D);  xn = scale * x / (norm + eps)
    gvf  = xn @ w_in.T                       # [L, 3D] -> g, v, f
    g = sigmoid(g); f = sigmoid(f - 1)
    h_t = f_t * h_{t-1} + (1 - f_t) * v_t    # sequential scan over L
    y = x + (g * h) @ w_out.T

Sharding: data-parallel over batch B=8 across the 8 NeuronCores (the
recurrence is independent per batch row); w_in/w_out/scale replicated.

Per-core dataflow (channels-on-partitions "transposed" layout for the
matmuls and the scan; the scan runs on the DVE TensorTensorScanArith
instruction with D on partitions and L on the free dim):
  - x streamed in natural [l, d] layout; RMSNorm stats on ACT (square
    +accum), xn = x*rinv on DVE (fp16), PE-transposed to [d, l] and
    evicted twice: fp16 (DVE) for the v matmuls and fp8 e4m3 (ACT)
    for the g/f matmuls.
  - proj_in: f and g projections run fp8 e4m3 with DoubleRow perf
    mode (2 k-tiles per matmul, 2x fp16 throughput); v stays fp16
    (its values pass straight through the scan into the output, so
    fp8 there would blow the error budget).  Measured end-to-end
    max-rel error 1.6e-2 vs the 2e-2 gate, dominated by these fp8
    paths (fp16-everything measures 2.3e-4).
  - f sigmoid on ACT in fp32 (the scan coefficient (f-1) would lose
    2^-11 absolute from an fp16 f, amplified 1/(1-f) by the
    recurrence); a = (f-1)*v fused on one DVE scalar_tensor_tensor;
    scan computes h = f*h - a, chained across L-chunks via `initial`.
  - proj_out: y = x + (g*h) @ w_out in natural layout, gh (fp8)
    DoubleRow-stationary, w_out8T streaming, PSUM in [128,512]
    halves (3 banks) so ps_gvf gets 4 banks; residual-add on DVE
    straight off PSUM.

Emission is software-pipelined for the in-order engine queues: x is
prefetched two chunks ahead and chunk c+1's norm/xn work is emitted
before chunk c's gate/scan work so the PE never waits on the DVE at
chunk boundaries.  Weight prep (fp16 conversion + PE transpose of
w_in/w_out) is interleaved with chunk 0: channel groups 0-3 of proj_in
only need w_in e-groups {0,2,4}, groups 4-7 need {1,3,5}, and w_out is
only needed by proj_out.
"""

import numpy as np
from contextlib import ExitStack

import concourse.bass as bass
import concourse.tile as tile
from concourse import bacc, mybir
from concourse.bass_utils import run_bass_kernel_spmd
from concourse.masks import make_identity

FP32 = mybir.dt.float32
FP16 = mybir.dt.float16
FP8 = mybir.dt.float8e4
MM_DR = mybir.MatmulPerfMode.DoubleRow

B, L, D = 8, 4096, 1024
E3 = 3 * D                 # 3072
LC = 512                   # L-chunk (PSUM bank free size in fp32)
NCH = L // LC              # 8 chunks
NLT = LC // 128            # 4 l-tiles per chunk
DK = D // 128              # 8 d-chunks (contraction tiles)
ME = E3 // 128             # 24 proj_in m-tiles
MO = D // 128              # 8 proj_out m-tiles
EPS = 1e-6
N_CORES = 8

AL = mybir.AluOpType
AF = mybir.ActivationFunctionType


def _emit(nc, nch=NCH):
    x_ap = nc.dram_tensor("x", [L, D], FP32, kind="ExternalInput").ap()
    w_in_ap = nc.dram_tensor("w_in", [E3, D], FP32, kind="ExternalInput").ap()
    w_out_ap = nc.dram_tensor("w_out", [D, D], FP32, kind="ExternalInput").ap()
    scale_ap = nc.dram_tensor("scale", [D], FP32, kind="ExternalInput").ap()
    y_ap = nc.dram_tensor("y", [L, D], FP32, kind="ExternalOutput").ap()

    with tile.TileContext(nc) as tc:
        with ExitStack() as ctx:
            # ---- persistent pools -------------------------------------
            wpool = ctx.enter_context(tc.tile_pool(name="weights", bufs=1))
            consts = ctx.enter_context(tc.tile_pool(name="consts", bufs=1))
            xpool = ctx.enter_context(tc.tile_pool(name="x", bufs=10))
            sqpool = ctx.enter_context(tc.tile_pool(name="sq", bufs=2))
            npool = ctx.enter_context(tc.tile_pool(name="norm", bufs=16))
            xnpool = ctx.enter_context(tc.tile_pool(name="xn", bufs=2))
            big = ctx.enter_context(tc.tile_pool(name="big", bufs=2))
            gates = ctx.enter_context(tc.tile_pool(name="gates", bufs=2))
            ypool = ctx.enter_context(tc.tile_pool(name="y", bufs=3))

            # PSUM: 3 + 3 + 2 = 8 banks.  ps16 is shared by weight-prep
            # transposes, xn transposes and y transposes (tag "tps").
            ps16 = ctx.enter_context(
                tc.tile_pool(name="ps16", bufs=1, space="PSUM"))
            ps_gvf = ctx.enter_context(
                tc.tile_pool(name="ps_gvf", bufs=5, space="PSUM"))
            ps_yn = ctx.enter_context(
                tc.tile_pool(name="ps_yn", bufs=2, space="PSUM"))

            ident16 = consts.tile([128, 128], FP16)
            make_identity(nc, ident16)
            bias_m1 = consts.tile([128, 1], FP32)
            nc.vector.memset(bias_m1[:], -1.0)

            # Weight layouts (all pre-scaled by scale[d], d on partitions):
            #  - g and f projections run in fp8 e4m3 with DoubleRow perf
            #    mode (2 k-tiles per matmul, 0.5 cycles/row):
            #    w8T[kp]: [128(d), 2(j), 2048(e)] fp8 where k = 2*kp+j and
            #    e 0:1024 is the g block, 1024:2048 the f block.
            #  - v projection stays fp16 (its values flow straight into
            #    the output; fp8 there would blow the error budget):
            #    w_inT_v[k]: [128(d), 1024(e)] fp16.
            #  - w_outT[k]: [128(d), 1024(e')] fp16.
            w8T = [wpool.tile([128, 2, 2 * D], FP8, tag=f"w8{kp}",
                              name=f"w8{kp}") for kp in range(DK // 2)]
            w_inT_v = [wpool.tile([128, D], FP16, tag=f"winv{k}",
                                  name=f"winv{k}") for k in range(DK)]
            w_out8T = [wpool.tile([128, 2, D], FP8, tag=f"wo8{kp}",
                                  name=f"wo8{kp}") for kp in range(DK // 2)]

            # ---- x prefetch + norm stages -----------------------------
            def stage_load(c):
                xs = []
                for i in range(NLT):
                    l0 = c * LC + 128 * i
                    xt = xpool.tile([128, D], FP32, tag="x")
                    nc.sync.dma_start(xt[:], x_ap[l0:l0 + 128, :])
                    xs.append(xt)
                return xs

            def stage_norm(xs):
                """RMSNorm stats + xn (fp16, on ACT). Returns xn tiles."""
                xns = []
                for i in range(NLT):
                    sq = sqpool.tile([128, D], FP16, tag="sq")
                    ssq = npool.tile([128, 1], FP32, tag="ssq")
                    nc.scalar.activation(sq[:], xs[i][:], AF.Square,
                                         accum_out=ssq[:])
                    nrm = npool.tile([128, 1], FP32, tag="nrm")
                    nc.scalar.activation(nrm[:], ssq[:], AF.Sqrt, scale=1.0 / D)
                    nc.vector.tensor_scalar_add(nrm[:], nrm[:], EPS)
                    rinv = npool.tile([128, 1], FP32, tag="rinv")
                    nc.vector.reciprocal(rinv[:], nrm[:])
                    xn = xnpool.tile([128, D], FP16, tag="xn")
                    nc.vector.tensor_scalar_mul(xn[:], xs[i][:], rinv[:])
                    xns.append(xn)
                return xns

            # ---- weight prep ------------------------------------------
            wprep_cm = tc.tile_pool(name="wprep", bufs=2)
            wprep = wprep_cm.__enter__()
            scale_row = wprep.tile([128, D], FP32, tag="srow", bufs=1)
            nc.gpsimd.dma_start(
                out=scale_row[:],
                in_=bass.AP(tensor=scale_ap.tensor, offset=scale_ap.offset,
                            ap=[[0, 128], [1, D]]))
            evict_flip = [0]

            def prep(src_ap, dst, do_scale, egs):
                """dst=None routes w_in egs into w8T (g/f, fp8) or
                w_inT_v (v, fp16); dst routes w_out egs."""
                for eg in egs:
                    w16s = []
                    for j in range(4):
                        e0 = (4 * eg + j) * 128
                        wt = wprep.tile([128, D], FP32, tag="wt", bufs=3)
                        nc.sync.dma_start(wt[:], src_ap[e0:e0 + 128, :])
                        w16 = wprep.tile([128, D], FP16, tag="w16", bufs=5)
                        if do_scale:
                            nc.vector.tensor_mul(w16[:], wt[:], scale_row[:])
                        else:
                            nc.scalar.copy(w16[:], wt[:])
                        w16s.append(w16)
                    for k in range(DK):
                        pst = ps16.tile([128, 512], FP16, tag="tps")
                        for j in range(4):
                            nc.tensor.transpose(
                                pst[:, 128 * j:128 * (j + 1)],
                                w16s[j][:, 128 * k:128 * (k + 1)],
                                ident16[:])
                        if dst is not None:
                            dstap = dst(k, eg)
                        elif eg < 2:        # g block -> fp8 pairs
                            dstap = w8T[k // 2][:, k % 2,
                                               512 * eg:512 * eg + 512]
                        elif eg < 4:        # v block -> fp16
                            dstap = w_inT_v[k][:, 512 * (eg - 2):
                                               512 * (eg - 2) + 512]
                        else:               # f block -> fp8 pairs at e+1024
                            dstap = w8T[k // 2][:, k % 2,
                                               1024 + 512 * (eg - 4):
                                               1024 + 512 * (eg - 4) + 512]
                        # alternate evictions between DVE and ACT
                        if evict_flip[0] % 2 == 0:
                            nc.vector.tensor_copy(dstap, pst[:])
                        else:
                            nc.scalar.copy(dstap, pst[:])
                        evict_flip[0] += 1

            # ---- per-chunk stages (emission is software-pipelined) ----
            def stage_transpose_xn(xns):
                """PE-transpose xn tiles into xnT [128(d), 8(k), 512(l)]
                fp16 (DVE evict), then one big ACT copy per chunk casts
                the block to fp8 for the DoubleRow g/f matmuls."""
                xnT = big.tile([128, DK, LC], FP16, tag="xnT")
                xn8T = big.tile([128, DK, LC], FP8, tag="xn8T")
                for i in range(NLT):
                    pst = ps16.tile([128, D], FP16, tag="tps")
                    for k in range(DK):
                        nc.tensor.transpose(
                            pst[:, 128 * k:128 * (k + 1)],
                            xns[i][:, 128 * k:128 * (k + 1)],
                            ident16[:])
                    src = pst[:].rearrange("p (k j) -> p k j", k=DK)
                    nc.vector.tensor_copy(
                        xnT[:, :, 128 * i:128 * (i + 1)], src)
                    # fp8 copy reads the SBUF fp16 tile, not PSUM: the
                    # ps16 bank is released by the DVE evict alone, so
                    # the next transpose group starts sooner.  The copy
                    # is only needed by next chunk's g/f matmuls, so the
                    # first two tiles ride the (otherwise idle, slow)
                    # GPSIMD to take load off ACT.
                    if i < 2:
                        nc.gpsimd.tensor_copy(
                            xn8T[:, :, 128 * i:128 * (i + 1)],
                            xnT[:, :, 128 * i:128 * (i + 1)])
                    else:
                        nc.scalar.copy(
                            xn8T[:, :, 128 * i:128 * (i + 1)],
                            xnT[:, :, 128 * i:128 * (i + 1)])
                return xnT, xn8T

            def stage_proj_in_gates(xnT, xn8T, h_prev, h, gh, cgs):
                """proj_in matmuls + gates + scan + g*h for channel groups.
                f and g run fp8 DoubleRow (4 k-pair matmuls); v runs fp16.
                f is computed first since its sigmoid is the first
                consumer, then v (feeds the STT), then g."""
                for cg in cgs:
                    pf = ps_gvf.tile([128, LC], FP32, tag="gvf")
                    pv = ps_gvf.tile([128, LC], FP32, tag="gvf")
                    pg = ps_gvf.tile([128, LC], FP32, tag="gvf")
                    for kp in range(DK // 2):
                        nc.tensor.matmul(
                            pf[:],
                            w8T[kp][:, :, 1024 + 128 * cg:1024 + 128 * (cg + 1)],
                            xn8T[:, 2 * kp:2 * kp + 2, :],
                            start=(kp == 0), stop=(kp == DK // 2 - 1),
                            perf_mode=MM_DR)
                    for k in range(DK):
                        nc.tensor.matmul(
                            pv[:], w_inT_v[k][:, 128 * cg:128 * (cg + 1)],
                            xnT[:, k, :],
                            start=(k == 0), stop=(k == DK - 1))
                    for kp in range(DK // 2):
                        nc.tensor.matmul(
                            pg[:],
                            w8T[kp][:, :, 128 * cg:128 * (cg + 1)],
                            xn8T[:, 2 * kp:2 * kp + 2, :],
                            start=(kp == 0), stop=(kp == DK // 2 - 1),
                            perf_mode=MM_DR)
                    # ft kept fp32: the scan coefficient (f-1) would lose
                    # ~2^-11 absolute from an fp16 f, and the recurrence
                    # amplifies that by 1/(1-f).
                    ft = gates.tile([128, LC], FP32, tag="f")
                    nc.scalar.activation(ft[:], pf[:], AF.Sigmoid,
                                         bias=bias_m1[:])
                    # an = (f - 1) * v == -(1-f)*v; scan then uses
                    # h = (f * h) - an = f*h + (1-f)*v.
                    at = gates.tile([128, LC], FP16, tag="a")
                    nc.vector.scalar_tensor_tensor(
                        at[:], ft[:], 1.0, pv[:], AL.subtract, AL.mult)
                    gt = gates.tile([128, LC], FP16, tag="g")
                    nc.scalar.activation(gt[:], pg[:], AF.Sigmoid)
                    init = 0.0 if h_prev is None else h_prev[:, cg, LC - 1:LC]
                    nc.vector.tensor_tensor_scan(
                        h[:, cg, :], ft[:], at[:], init, AL.mult, AL.subtract)
                    nc.vector.tensor_mul(gh[:, cg, :], gt[:], h[:, cg, :])

            def stage_out(c, gh, xs):
                """proj_out directly in natural layout: gh slices are the
                stationary operand, w_outT streams.  The residual x is
                preloaded into PSUM by ACT; matmuls accumulate on top
                (start=False), so the bank already holds y and eviction is
                a plain ACT copy (no DVE work)."""
                for i in range(NLT):
                    l0 = c * LC + 128 * i
                    ys = ypool.tile([128, D], FP32, tag="y")
                    for half in range(2):
                        e0 = 512 * half
                        pyn = ps_yn.tile([128, 512], FP32, tag="yn")
                        for kp in range(DK // 2):
                            lhsT = gh[:, 2 * kp:2 * kp + 2,
                                      128 * i:128 * (i + 1)]
                            nc.tensor.matmul(
                                pyn[:], lhsT,
                                w_out8T[kp][:, :, e0:e0 + 512],
                                start=(kp == 0), stop=(kp == DK // 2 - 1),
                                perf_mode=MM_DR)
                        nc.vector.tensor_add(
                            ys[:, e0:e0 + 512], pyn[:],
                            xs[i][:, e0:e0 + 512])
                        nc.sync.dma_start(
                            y_ap[l0:l0 + 128, e0:e0 + 512],
                            ys[:, e0:e0 + 512])

            def new_h_gh():
                h = big.tile([128, DK, LC], FP16, tag="h", name="h")
                gh = big.tile([128, DK, LC], FP8, tag="gh", bufs=1, name="gh")
                return h, gh

            # ---- chunk 0, interleaved with weight prep ----------------
            xs_pre = {}
            prep(w_in_ap, None, True, [0])
            xs_pre[0] = stage_load(0)
            prep(w_in_ap, None, True, [2, 4])       # [0,2,4] unlock cg 0-3
            xs_pre[1] = stage_load(1)
            xns = stage_norm(xs_pre[0])
            xnT, xn8T = stage_transpose_xn(xns)
            xns_n = stage_norm(xs_pre[1])
            h, gh = new_h_gh()
            stage_proj_in_gates(xnT, xn8T, None, h, gh, range(0, 4))
            prep(w_in_ap, None, True, [1, 3, 5])    # unlocks cg 4-7
            stage_proj_in_gates(xnT, xn8T, None, h, gh, range(4, 8))
            prep(w_out_ap,
                 lambda k, eg: w_out8T[k // 2][:, k % 2,
                                              512 * eg:512 * eg + 512],
                 False, [0, 1])
            wprep_cm.__exit__(None, None, None)
            h_prev = h
            xnT, xn8T = stage_transpose_xn(xns_n)
            xs_pre[2] = stage_load(2)
            stage_out(0, gh, xs_pre[0])

            # ---- steady-state chunks ----------------------------------
            for c in range(1, nch):
                if c + 1 < nch:
                    xns_n = stage_norm(xs_pre[c + 1])
                h, gh = new_h_gh()
                stage_proj_in_gates(xnT, xn8T, h_prev, h, gh, range(DK))
                h_prev = h
                if c + 1 < nch:
                    xnT, xn8T = stage_transpose_xn(xns_n)
                if c + 2 < nch:
                    xs_pre[c + 2] = stage_load(c + 2)
                stage_out(c, gh, xs_pre[c])

    nc.compile()
    return nc


_NC_CACHE = None


def _get_nc():
    global _NC_CACHE
    if _NC_CACHE is None:
        nc = bacc.Bacc("TRN2", target_bir_lowering=False, debug=False)
        _NC_CACHE = _emit(nc)
    return _NC_CACHE


def _run(inputs, **kw):
    x = np.ascontiguousarray(inputs["x"], dtype=np.float32)
    w_in = np.ascontiguousarray(inputs["w_in"], dtype=np.float32)
    w_out = np.ascontiguousarray(inputs["w_out"], dtype=np.float32)
    scale = np.ascontiguousarray(inputs["scale"], dtype=np.float32)
    nc = _get_nc()
    in_maps = [
        {"x": x[b], "w_in": w_in, "w_out": w_out, "scale": scale}
        for b in range(B)
    ]
    res = run_bass_kernel_spmd(nc, in_maps, list(range(N_CORES)), **kw)
    out = np.stack([res.results[b]["y"] for b in range(B)], axis=0)
    return out, res


def kernel(**inputs) -> np.ndarray:
    out, _ = _run(inputs)
    return out



# revision 44
# speedup vs baseline: 1.0670x; 1.0670x over previous
"""Trainium2 Bass kernel for nn_LinearRecurrenceLayer.

Reference computation (per batch row, L=4096, D=1024):
    norm = ||x_l|| / sqrt(D);  xn = scale * x / (norm + eps)
    gvf  = xn @ w_in.T                       # [L, 3D] -> g, v, f
    g = sigmoid(g); f = sigmoid(f - 1)
    h_t = f_t * h_{t-1} + (1 - f_t) * v_t    # sequential scan over L
    y = x + (g * h) @ w_out.T

Sharding: data-parallel over batch B=8 across the 8 NeuronCores (the
recurrence is independent per batch row); w_in/w_out/scale replicated.

Per-core dataflow (channels-on-partitions "transposed" layout for the
matmuls and the scan; the scan runs on the DVE TensorTensorScanArith
instruction with D on partitions and L on the free dim):
  - x streamed in natural [l, d] layout; RMSNorm stats on ACT (square
    +accum), xn = x*rinv on DVE (fp16), PE-transposed to [d, l] and
    evicted twice: fp16 (DVE) for the v matmuls and fp8 e4m3 (ACT)
    for the g/f matmuls.
  - proj_in: f and g projections run fp8 e4m3 with DoubleRow perf
    mode (2 k-tiles per matmul, 2x fp16 throughput); v stays fp16
    (its values pass straight through the scan into the output, so
    fp8 there would blow the error budget).  Measured end-to-end
    max-rel error 1.6e-2 vs the 2e-2 gate, dominated by these fp8
    paths (fp16-everything measures 2.3e-4).
  - f sigmoid on ACT in fp32 (the scan coefficient (f-1) would lose
    2^-11 absolute from an fp16 f, amplified 1/(1-f) by the
    recurrence); a = (f-1)*v fused on one DVE scalar_tensor_tensor;
    scan computes h = f*h - a, chained across L-chunks via `initial`.
  - proj_out: y = x + (g*h) @ w_out in natural layout, gh (fp8)
    DoubleRow-stationary, w_out8T streaming, PSUM in [128,512]
    halves (3 banks) so ps_gvf gets 4 banks; residual-add on DVE
    straight off PSUM.

Emission is software-pipelined for the in-order engine queues: x is
prefetched two chunks ahead and chunk c+1's norm/xn work is emitted
before chunk c's gate/scan work so the PE never waits on the DVE at
chunk boundaries.  Weight prep (fp16 conversion + PE transpose of
w_in/w_out) is interleaved with chunk 0: channel groups 0-3 of proj_in
only need w_in e-groups {0,2,4}, groups 4-7 need {1,3,5}, and w_out is
only needed by proj_out.
"""

import numpy as np
from contextlib import ExitStack

import concourse.bass as bass
import concourse.tile as tile
from concourse import bacc, mybir
from concourse.bass_utils import run_bass_kernel_spmd
from concourse.masks import make_identity

FP32 = mybir.dt.float32
FP16 = mybir.dt.float16
FP8 = mybir.dt.float8e4
MM_DR = mybir.MatmulPerfMode.DoubleRow

B, L, D = 8, 4096, 1024
E3 = 3 * D                 # 3072
LC = 512                   # L-chunk (PSUM bank free size in fp32)
NCH = L // LC              # 8 chunks
NLT = LC // 128            # 4 l-tiles per chunk
DK = D // 128              # 8 d-chunks (contraction tiles)
ME = E3 // 128             # 24 proj_in m-tiles
MO = D // 128              # 8 proj_out m-tiles
EPS = 1e-6
N_CORES = 8

AL = mybir.AluOpType
AF = mybir.ActivationFunctionType


def _emit(nc, nch=NCH):
    x_ap = nc.dram_tensor("x", [L, D], FP32, kind="ExternalInput").ap()
    w_in_ap = nc.dram_tensor("w_in", [E3, D], FP32, kind="ExternalInput").ap()
    w_out_ap = nc.dram_tensor("w_out", [D, D], FP32, kind="ExternalInput").ap()
    scale_ap = nc.dram_tensor("scale", [D], FP32, kind="ExternalInput").ap()
    y_ap = nc.dram_tensor("y", [L, D], FP32, kind="ExternalOutput").ap()

    with tile.TileContext(nc) as tc:
        with ExitStack() as ctx:
            # ---- persistent pools -------------------------------------
            wpool = ctx.enter_context(tc.tile_pool(name="weights", bufs=1))
            consts = ctx.enter_context(tc.tile_pool(name="consts", bufs=1))
            xpool = ctx.enter_context(tc.tile_pool(name="x", bufs=10))
            sqpool = ctx.enter_context(tc.tile_pool(name="sq", bufs=2))
            npool = ctx.enter_context(tc.tile_pool(name="norm", bufs=16))
            xnpool = ctx.enter_context(tc.tile_pool(name="xn", bufs=4))
            big = ctx.enter_context(tc.tile_pool(name="big", bufs=2))
            gates = ctx.enter_context(tc.tile_pool(name="gates", bufs=3))
            ypool = ctx.enter_context(tc.tile_pool(name="y", bufs=2))

            # PSUM: 3 + 3 + 2 = 8 banks.  ps16 is shared by weight-prep
            # transposes, xn transposes and y transposes (tag "tps").
            ps16 = ctx.enter_context(
                tc.tile_pool(name="ps16", bufs=1, space="PSUM"))
            ps_gvf = ctx.enter_context(
                tc.tile_pool(name="ps_gvf", bufs=5, space="PSUM"))
            ps_yn = ctx.enter_context(
                tc.tile_pool(name="ps_yn", bufs=2, space="PSUM"))

            ident16 = consts.tile([128, 128], FP16)
            make_identity(nc, ident16)
            bias_m1 = consts.tile([128, 1], FP32)
            nc.vector.memset(bias_m1[:], -1.0)

            # Weight layouts (all pre-scaled by scale[d], d on partitions):
            #  - g and f projections run in fp8 e4m3 with DoubleRow perf
            #    mode (2 k-tiles per matmul, 0.5 cycles/row):
            #    w8T[kp]: [128(d), 2(j), 2048(e)] fp8 where k = 2*kp+j and
            #    e 0:1024 is the g block, 1024:2048 the f block.
            #  - v projection stays fp16 (its values flow straight into
            #    the output; fp8 there would blow the error budget):
            #    w_inT_v[k]: [128(d), 1024(e)] fp16.
            #  - w_outT[k]: [128(d), 1024(e')] fp16.
            w8T = [wpool.tile([128, 2, 2 * D], FP8, tag=f"w8{kp}",
                              name=f"w8{kp}") for kp in range(DK // 2)]
            w_inT_v = [wpool.tile([128, D], FP16, tag=f"winv{k}",
                                  name=f"winv{k}") for k in range(DK)]
            w_out8T = [wpool.tile([128, 2, D], FP8, tag=f"wo8{kp}",
                                  name=f"wo8{kp}") for kp in range(DK // 2)]

            # ---- x prefetch + norm stages -----------------------------
            def stage_load(c):
                xs = []
                for i in range(NLT):
                    l0 = c * LC + 128 * i
                    xt = xpool.tile([128, D], FP32, tag="x")
                    nc.sync.dma_start(xt[:], x_ap[l0:l0 + 128, :])
                    xs.append(xt)
                return xs

            def stage_norm(xs):
                """RMSNorm stats + xn (fp16, on ACT). Returns xn tiles."""
                xns = []
                for i in range(NLT):
                    sq = sqpool.tile([128, D], FP16, tag="sq")
                    ssq = npool.tile([128, 1], FP32, tag="ssq")
                    nc.scalar.activation(sq[:], xs[i][:], AF.Square,
                                         accum_out=ssq[:])
                    nrm = npool.tile([128, 1], FP32, tag="nrm")
                    nc.scalar.activation(nrm[:], ssq[:], AF.Sqrt, scale=1.0 / D)
                    nc.vector.tensor_scalar_add(nrm[:], nrm[:], EPS)
                    rinv = npool.tile([128, 1], FP32, tag="rinv")
                    nc.vector.reciprocal(rinv[:], nrm[:])
                    xn = xnpool.tile([128, D], FP16, tag="xn")
                    nc.vector.tensor_scalar_mul(xn[:], xs[i][:], rinv[:])
                    xns.append(xn)
                return xns

            # ---- weight prep ------------------------------------------
            wprep_cm = tc.tile_pool(name="wprep", bufs=2)
            wprep = wprep_cm.__enter__()
            scale_row = wprep.tile([128, D], FP32, tag="srow", bufs=1)
            nc.gpsimd.dma_start(
                out=scale_row[:],
                in_=bass.AP(tensor=scale_ap.tensor, offset=scale_ap.offset,
                            ap=[[0, 128], [1, D]]))
            evict_flip = [0]

            def prep(src_ap, dst, do_scale, egs):
                """dst=None routes w_in egs into w8T (g/f, fp8) or
                w_inT_v (v, fp16); dst routes w_out egs."""
                for eg in egs:
                    w16s = []
                    for j in range(4):
                        e0 = (4 * eg + j) * 128
                        wt = wprep.tile([128, D], FP32, tag="wt", bufs=3)
                        nc.sync.dma_start(wt[:], src_ap[e0:e0 + 128, :])
                        w16 = wprep.tile([128, D], FP16, tag="w16", bufs=5)
                        if do_scale:
                            nc.vector.tensor_mul(w16[:], wt[:], scale_row[:])
                        else:
                            nc.scalar.copy(w16[:], wt[:])
                        w16s.append(w16)
                    for k in range(DK):
                        pst = ps16.tile([128, 512], FP16, tag="tps")
                        for j in range(4):
                            nc.tensor.transpose(
                                pst[:, 128 * j:128 * (j + 1)],
                                w16s[j][:, 128 * k:128 * (k + 1)],
                                ident16[:])
                        if dst is not None:
                            dstap = dst(k, eg)
                        elif eg < 2:        # g block -> fp8 pairs
                            dstap = w8T[k // 2][:, k % 2,
                                               512 * eg:512 * eg + 512]
                        elif eg < 4:        # v block -> fp16
                            dstap = w_inT_v[k][:, 512 * (eg - 2):
                                               512 * (eg - 2) + 512]
                        else:               # f block -> fp8 pairs at e+1024
                            dstap = w8T[k // 2][:, k % 2,
                                               1024 + 512 * (eg - 4):
                                               1024 + 512 * (eg - 4) + 512]
                        # alternate evictions between DVE and ACT
                        if evict_flip[0] % 2 == 0:
                            nc.vector.tensor_copy(dstap, pst[:])
                        else:
                            nc.scalar.copy(dstap, pst[:])
                        evict_flip[0] += 1

            # ---- per-chunk stages (emission is software-pipelined) ----
            def stage_transpose_xn(xns):
                """PE-transpose xn tiles into xnT [128(d), 8(k), 512(l)]
                fp16 (DVE evict), then one big ACT copy per chunk casts
                the block to fp8 for the DoubleRow g/f matmuls."""
                xnT = big.tile([128, DK, LC], FP16, tag="xnT")
                xn8T = big.tile([128, DK, LC], FP8, tag="xn8T")
                for i in range(NLT):
                    pst = ps16.tile([128, D], FP16, tag="tps")
                    for k in range(DK):
                        nc.tensor.transpose(
                            pst[:, 128 * k:128 * (k + 1)],
                            xns[i][:, 128 * k:128 * (k + 1)],
                            ident16[:])
                    src = pst[:].rearrange("p (k j) -> p k j", k=DK)
                    nc.vector.tensor_copy(
                        xnT[:, :, 128 * i:128 * (i + 1)], src)
                    # fp8 copy reads the SBUF fp16 tile, not PSUM: the
                    # ps16 bank is released by the DVE evict alone, so
                    # the next transpose group starts sooner.
                    nc.scalar.copy(
                        xn8T[:, :, 128 * i:128 * (i + 1)],
                        xnT[:, :, 128 * i:128 * (i + 1)])
                return xnT, xn8T

            def stage_proj_in_gates(xnT, xn8T, h_prev, h, gh, cgs):
                """proj_in matmuls + gates + scan + g*h for channel groups.
                f and g run fp8 DoubleRow (4 k-pair matmuls); v runs fp16.
                f is computed first since its sigmoid is the first
                consumer, then v (feeds the STT), then g."""
                for cg in cgs:
                    pf = ps_gvf.tile([128, LC], FP32, tag="gvf")
                    pv = ps_gvf.tile([128, LC], FP32, tag="gvf")
                    pg = ps_gvf.tile([128, LC], FP32, tag="gvf")
                    for kp in range(DK // 2):
                        nc.tensor.matmul(
                            pf[:],
                            w8T[kp][:, :, 1024 + 128 * cg:1024 + 128 * (cg + 1)],
                            xn8T[:, 2 * kp:2 * kp + 2, :],
                            start=(kp == 0), stop=(kp == DK // 2 - 1),
                            perf_mode=MM_DR)
                    for k in range(DK):
                        nc.tensor.matmul(
                            pv[:], w_inT_v[k][:, 128 * cg:128 * (cg + 1)],
                            xnT[:, k, :],
                            start=(k == 0), stop=(k == DK - 1))
                    for kp in range(DK // 2):
                        nc.tensor.matmul(
                            pg[:],
                            w8T[kp][:, :, 128 * cg:128 * (cg + 1)],
                            xn8T[:, 2 * kp:2 * kp + 2, :],
                            start=(kp == 0), stop=(kp == DK // 2 - 1),
                            perf_mode=MM_DR)
                    # ft kept fp32: the scan coefficient (f-1) would lose
                    # ~2^-11 absolute from an fp16 f, and the recurrence
                    # amplifies that by 1/(1-f).
                    ft = gates.tile([128, LC], FP32, tag="f")
                    nc.scalar.activation(ft[:], pf[:], AF.Sigmoid,
                                         bias=bias_m1[:])
                    # an = (f - 1) * v == -(1-f)*v; scan then uses
                    # h = (f * h) - an = f*h + (1-f)*v.
                    at = gates.tile([128, LC], FP16, tag="a")
                    nc.vector.scalar_tensor_tensor(
                        at[:], ft[:], 1.0, pv[:], AL.subtract, AL.mult)
                    gt = gates.tile([128, LC], FP16, tag="g")
                    nc.scalar.activation(gt[:], pg[:], AF.Sigmoid)
                    init = 0.0 if h_prev is None else h_prev[:, cg, LC - 1:LC]
                    nc.vector.tensor_tensor_scan(
                        h[:, cg, :], ft[:], at[:], init, AL.mult, AL.subtract)
                    nc.vector.tensor_mul(gh[:, cg, :], gt[:], h[:, cg, :])

            def stage_out(c, gh, xs):
                """proj_out directly in natural layout: gh slices are the
                stationary operand, w_outT streams.  The residual x is
                preloaded into PSUM by ACT; matmuls accumulate on top
                (start=False), so the bank already holds y and eviction is
                a plain ACT copy (no DVE work)."""
                for i in range(NLT):
                    l0 = c * LC + 128 * i
                    ys = ypool.tile([128, D], FP32, tag="y")
                    for half in range(2):
                        e0 = 512 * half
                        pyn = ps_yn.tile([128, 512], FP32, tag="yn")
                        for kp in range(DK // 2):
                            lhsT = gh[:, 2 * kp:2 * kp + 2,
                                      128 * i:128 * (i + 1)]
                            nc.tensor.matmul(
                                pyn[:], lhsT,
                                w_out8T[kp][:, :, e0:e0 + 512],
                                start=(kp == 0), stop=(kp == DK // 2 - 1),
                                perf_mode=MM_DR)
                        nc.vector.tensor_add(
                            ys[:, e0:e0 + 512], pyn[:],
                            xs[i][:, e0:e0 + 512])
                        nc.sync.dma_start(
                            y_ap[l0:l0 + 128, e0:e0 + 512],
                            ys[:, e0:e0 + 512])

            def new_h_gh():
                h = big.tile([128, DK, LC], FP16, tag="h", name="h")
                gh = big.tile([128, DK, LC], FP8, tag="gh", bufs=1, name="gh")
                return h, gh

            # ---- chunk 0, interleaved with weight prep ----------------
            xs_pre = {}
            prep(w_in_ap, None, True, [0])
            xs_pre[0] = stage_load(0)
            prep(w_in_ap, None, True, [2, 4])       # [0,2,4] unlock cg 0-3
            xs_pre[1] = stage_load(1)
            xns = stage_norm(xs_pre[0])
            xnT, xn8T = stage_transpose_xn(xns)
            xns_n = stage_norm(xs_pre[1])
            h, gh = new_h_gh()
            stage_proj_in_gates(xnT, xn8T, None, h, gh, range(0, 4))
            prep(w_in_ap, None, True, [1, 3, 5])    # unlocks cg 4-7
            stage_proj_in_gates(xnT, xn8T, None, h, gh, range(4, 8))
            prep(w_out_ap,
                 lambda k, eg: w_out8T[k // 2][:, k % 2,
                                              512 * eg:512 * eg + 512],
                 False, [0, 1])
            wprep_cm.__exit__(None, None, None)
            h_prev = h
            xnT, xn8T = stage_transpose_xn(xns_n)
            xs_pre[2] = stage_load(2)
            stage_out(0, gh, xs_pre[0])

            # ---- steady-state chunks ----------------------------------
            for c in range(1, nch):
                if c + 1 < nch:
                    xns_n = stage_norm(xs_pre[c + 1])
                h, gh = new_h_gh()
                stage_proj_in_gates(xnT, xn8T, h_prev, h, gh, range(DK))
                h_prev = h
                if c + 1 < nch:
                    xnT, xn8T = stage_transpose_xn(xns_n)
                if c + 2 < nch:
                    xs_pre[c + 2] = stage_load(c + 2)
                stage_out(c, gh, xs_pre[c])

    nc.compile()
    return nc


_NC_CACHE = None


def _get_nc():
    global _NC_CACHE
    if _NC_CACHE is None:
        nc = bacc.Bacc("TRN2", target_bir_lowering=False, debug=False)
        _NC_CACHE = _emit(nc)
    return _NC_CACHE


def _run(inputs, **kw):
    x = np.ascontiguousarray(inputs["x"], dtype=np.float32)
    w_in = np.ascontiguousarray(inputs["w_in"], dtype=np.float32)
    w_out = np.ascontiguousarray(inputs["w_out"], dtype=np.float32)
    scale = np.ascontiguousarray(inputs["scale"], dtype=np.float32)
    nc = _get_nc()
    in_maps = [
        {"x": x[b], "w_in": w_in, "w_out": w_out, "scale": scale}
        for b in range(B)
    ]
    res = run_bass_kernel_spmd(nc, in_maps, list(range(N_CORES)), **kw)
    out = np.stack([res.results[b]["y"] for b in range(B)], axis=0)
    return out, res


def kernel(**inputs) -> np.ndarray:
    out, _ = _run(inputs)
    return out

